# revision 5
# baseline (speedup 1.0000x reference)
"""Multi-head attention block on 8 Trainium2 NeuronCores (Bass/Tile).

Problem: x[4,2048,1024], per-head projections Mq/Mk[1024,8,64], Mv[1024,8,128]
  Q,K,V = per-head projections of x; out = softmax(QK^T/8) V, heads concat
  -> [4,2048,1024].

Sharding: 8 cores = 4 batches x 2 head-groups (4 heads each). Core c handles
batch b=c//2, heads [4g..4g+4) with g=c%2. No collectives; host gathers.

Per-core kernel (all matmuls fp32r, full PE rate at N=512):
  inputs (host pre-laid-out):
    xt  [1024, 2048] = x[b].T          (d on partitions -> no on-chip transpose)
    wqk [1024, 512]  = [Qh0|Qh1|Kh0|Kh1|Qh2|Qh3|Kh2|Kh3] col blocks (64 each)
    wv  [1024, 512]  = Mv heads packed (128 cols per head)
  stage 1: QT/KT per head-pair -> SBUF [128=2x64 dk, 2048 s]; V natural
           [128 sk, 4x128 dv] chunks.
  stage 2: scores^T[sk,sq] = KT^T... lhsT=KT-chunk, rhs=QT -> PSUM;
           exp(S^T/8) on ScalarE (no max subtraction: |scores/8| < ~6);
           AV: lhsT=V-chunk, rhs=W^T accumulate -> out^T[dv,sq];
           denominators via ones-matmul (column sums of W^T), broadcast to
           all partitions for free; reciprocal+scale on VectorE.
  output: outT [512, 2048] (4 heads x 128 dv, transposed); host transposes.
"""

import numpy as np
from contextlib import ExitStack

B, S, D = 4, 2048, 1024
H, DK, DV = 8, 64, 128
N_CORES = 8
HPC = 4          # heads per core
P = 128          # partitions
ND = D // P      # 8 d-chunks
NSQ = S // 512   # 4 query tiles of 512
NSK = S // P     # 16 key chunks of 128
CG = 2           # key chunks per exp batch (exp over [128, CG*512])

_CACHE = {}


def _build_nc():
    import concourse.tile as tile
    from concourse import bacc, mybir

    F32 = mybir.dt.float32
    F32R = mybir.dt.float32r
    EXP = mybir.ActivationFunctionType.Exp

    nc = bacc.Bacc("TRN2", target_bir_lowering=False, debug=False)
    xt_d = nc.dram_tensor("xt", [D, S], F32R, kind="ExternalInput")
    wqk_d = nc.dram_tensor("wqk", [D, HPC * DK * 2], F32R,
                            kind="ExternalInput")
    wv_d = nc.dram_tensor("wv", [D, HPC * DV], F32R, kind="ExternalInput")
    out_d = nc.dram_tensor("outT", [HPC * DV, S], F32, kind="ExternalOutput")

    with tile.TileContext(nc) as tc, ExitStack() as ctx:
        const_p = ctx.enter_context(tc.tile_pool(name="const", bufs=1))
        xt_p = ctx.enter_context(tc.tile_pool(name="xt", bufs=1))
        w_p = ctx.enter_context(tc.tile_pool(name="w", bufs=1))
        qk_p = ctx.enter_context(tc.tile_pool(name="qk", bufs=1))
        v_p = ctx.enter_context(tc.tile_pool(name="v", bufs=1))
        wt_p = ctx.enter_context(tc.tile_pool(name="wt", bufs=3))
        out_p = ctx.enter_context(tc.tile_pool(name="out", bufs=2))
        rec_p = ctx.enter_context(tc.tile_pool(name="rec", bufs=2))
        import concourse.bass as bass

        psA = ctx.enter_context(
            tc.tile_pool(name="psA", bufs=2, space=bass.MemorySpace.PSUM))
        psB = ctx.enter_context(
            tc.tile_pool(name="psB", bufs=2, space=bass.MemorySpace.PSUM))
        psC = ctx.enter_context(
            tc.tile_pool(name="psC", bufs=2, space=bass.MemorySpace.PSUM))

        ones_f = const_p.tile([P, P], F32)
        nc.vector.memset(ones_f[:], 1.0)
        ones = const_p.tile([P, P], F32R)
        nc.vector.tensor_copy(ones[:], ones_f[:])

        xt_sb = xt_p.tile([P, ND * S], F32R)
        wqk_sb = w_p.tile([P, ND * 512], F32R)
        wv_sb = w_p.tile([P, ND * 512], F32R)
        for d in range(ND):
            nc.sync.dma_start(
                xt_sb[:, S * d:S * (d + 1)], xt_d[P * d:P * (d + 1), :])
            nc.sync.dma_start(
                wqk_sb[:, 512 * d:512 * (d + 1)], wqk_d[P * d:P * (d + 1), :])
            nc.sync.dma_start(
                wv_sb[:, 512 * d:512 * (d + 1)], wv_d[P * d:P * (d + 1), :])

        # stage 1a: QT/KT head-pair blocks. blk order: QQ0, KK0, QQ1, KK1.
        # qk_tiles[blk] [128, 2048]: partitions 0-63 = first head of pair,
        # 64-127 = second head.
        qk_tiles = [qk_p.tile([P, S], F32R, name=f"qkt{i}", tag=f"qk{i}")
                    for i in range(4)]
        for blk in range(4):
            for sq in range(NSQ):
                ps = psB.tile([P, 512], F32, name="ps1a", tag="b")
                for d in range(ND):
                    nc.tensor.matmul(
                        ps[:],
                        lhsT=wqk_sb[:, 512 * d + P * blk:
                                    512 * d + P * (blk + 1)],
                        rhs=xt_sb[:, S * d + 512 * sq:
                                  S * d + 512 * (sq + 1)],
                        start=(d == 0), stop=(d == ND - 1))
                nc.vector.tensor_copy(
                    qk_tiles[blk][:, 512 * sq:512 * (sq + 1)], ps[:])

        # stage 1b: V natural [sk, dv], all 4 heads packed along free dim.
        # vall[:, 512*c + 128*h + v] = V_h[128*c + p, v]
        vall = v_p.tile([P, NSK * 512], F32R)
        for c in range(NSK):
            ps = psC.tile([P, 512], F32, name="ps1b", tag="c")
            for d in range(ND):
                nc.tensor.matmul(
                    ps[:],
                    lhsT=xt_sb[:, S * d + P * c:
                               S * d + P * (c + 1)],
                    rhs=wv_sb[:, 512 * d:512 * (d + 1)],
                    start=(d == 0), stop=(d == ND - 1))
            nc.vector.tensor_copy(vall[:, 512 * c:512 * (c + 1)], ps[:])

        # stage 2: attention per (head, sq-tile)
        for h in range(HPC):
            pair, i = divmod(h, 2)
            qq = qk_tiles[2 * pair]
            kk = qk_tiles[2 * pair + 1]
            for sq in range(NSQ):
                ps_av = psB.tile([P, 512], F32, name="ps_av", tag="b")
                ps_sum = psC.tile([P, 512], F32, name="ps_sum", tag="c")
                for cg in range(NSK // CG):
                    ps_sc = psA.tile([P, CG * 512], F32, name="ps_sc", tag="a")
                    for cc in range(CG):
                        c = CG * cg + cc
                        nc.tensor.matmul(
                            ps_sc[:, 512 * cc:512 * (cc + 1)],
                            lhsT=kk[DK * i:DK * (i + 1),
                                    P * c:P * (c + 1)],
                            rhs=qq[DK * i:DK * (i + 1),
                                   512 * sq:512 * (sq + 1)],
                            start=True, stop=True)
                    wt = wt_p.tile([P, CG * 512], F32R)
                    nc.scalar.activation(wt[:], ps_sc[:], EXP, scale=0.125)
                    for cc in range(CG):
                        c = CG * cg + cc
                        nc.tensor.matmul(
                            ps_av[:],
                            lhsT=vall[:, 512 * c + P * h:
                                      512 * c + P * (h + 1)],
                            rhs=wt[:, 512 * cc:512 * (cc + 1)],
                            start=(c == 0), stop=(c == NSK - 1))
                        nc.tensor.matmul(
                            ps_sum[:],
                            lhsT=ones[:],
                            rhs=wt[:, 512 * cc:512 * (cc + 1)],
                            start=(c == 0), stop=(c == NSK - 1))
                rec = rec_p.tile([P, 512], F32)
                nc.vector.reciprocal(rec[:], ps_sum[:])
                o = out_p.tile([P, 512], F32)
                nc.vector.tensor_mul(o[:], ps_av[:], rec[:])
                nc.sync.dma_start(
                    out_d[P * h:P * (h + 1), 512 * sq:512 * (sq + 1)], o[:])

    nc.compile()
    return nc


def _core_inputs(x, Mq, Mk, Mv, core):
    b, g = divmod(core, 2)
    hs = [HPC * g + j for j in range(HPC)]
    xt = np.ascontiguousarray(np.asarray(x[b], dtype=np.float32).T)
    cols = []
    for pair in range(2):
        h0, h1 = hs[2 * pair], hs[2 * pair + 1]
        cols += [Mq[:, h0, :], Mq[:, h1, :], Mk[:, h0, :], Mk[:, h1, :]]
    wqk = np.ascontiguousarray(np.concatenate(cols, axis=1), dtype=np.float32)
    wv = np.ascontiguousarray(
        np.asarray(Mv[:, hs, :], dtype=np.float32).reshape(D, HPC * DV))
    return {"xt": xt, "wqk": wqk, "wv": wv}


def kernel(x, Mq, Mk, Mv):
    from concourse.bass_utils import run_bass_kernel_spmd

    x = np.asarray(x, dtype=np.float32)
    Mq = np.asarray(Mq, dtype=np.float32)
    Mk = np.asarray(Mk, dtype=np.float32)
    Mv = np.asarray(Mv, dtype=np.float32)

    if "nc" not in _CACHE:
        _CACHE["nc"] = _build_nc()
    nc = _CACHE["nc"]

    in_maps = [_core_inputs(x, Mq, Mk, Mv, c) for c in range(N_CORES)]
    res = run_bass_kernel_spmd(nc, in_maps, list(range(N_CORES))).results

    out = np.empty((B, S, H * DV), dtype=np.float32)
    for core in range(N_CORES):
        b, g = divmod(core, 2)
        out[b, :, 512 * g:512 * (g + 1)] = res[core]["outT"].T
    return out


# revision 10
# speedup vs baseline: 1.5526x; 1.5526x over previous
"""Multi-head attention block on 8 Trainium2 NeuronCores (Bass/Tile).

Problem: x[4,2048,1024], per-head projections Mq/Mk[1024,8,64], Mv[1024,8,128]
  Q,K,V = per-head projections of x; out = softmax(QK^T/8) V, heads concat
  -> [4,2048,1024].

Sharding: 8 cores = 4 batches x 2 head-groups (4 heads each). Core c handles
batch b=c//2, heads [4g..4g+4) with g=c%2. No collectives; host gathers.

Per-core kernel (all matmuls bf16 operands, fp32 PSUM accumulate):
  inputs (host pre-laid-out, bf16):
    xt  [1024, 2048] = x[b].T          (d on partitions -> no on-chip transpose)
    wqk [1024, 512]  = [Qh0|Qh1|Kh0|Kh1|Qh2|Qh3|Kh2|Kh3] col blocks (64 each)
    wv  [1024, 512]  = Mv heads packed (128 cols per head)
  stage 1: QT/KT per head-pair -> SBUF [128=2x64 dk, 2048 s]; V natural
           [128 sk, 4x128 dv] chunks.
  stage 2 per (head-pair, sq-tile), software-pipelined over the 16 sk-chunks:
    scores^T[sk,sq] for both heads of the pair in one PSUM tile
      (head0 at rows 0-63 of the PE array, head1 at rows 64-127 ->
       the two matmuls run concurrently and hide each other's LDWEIGHTS);
    exp((S^T)/8) on ScalarE over [128, 1024] (both heads at once; no max
      subtraction: |scores/8| < ~6);
    AV: lhsT=V-chunk, rhs=W^T accumulate -> out^T[dv,sq] UNNORMALIZED;
    denominators via ones-matmul column sums (M=1; head0 -> PSUM partition 0,
      head1 -> partition 64 = disjoint PE column groups, so they overlap).
    AV+sums of chunk c-1 are emitted after scores of chunk c so the PE
    stays busy while ScalarE runs exp.
  outputs: outT [512, 2048] unnormalized (4 heads x 128 dv, transposed) and
           sums [16, 512] = denominators per (head, sq-tile), DMA'd straight
           from PSUM. Host divides, transposes, concatenates.
"""

import numpy as np
from contextlib import ExitStack

B, S, D = 4, 2048, 1024
H, DK, DV = 8, 64, 128
N_CORES = 8
HPC = 4          # heads per core
P = 128          # partitions
ND = D // P      # 8 d-chunks
NSQ = S // 512   # 4 query tiles of 512
NSK = S // P     # 16 key chunks of 128

_CACHE = {}


def _build_nc():
    import concourse.tile as tile
    from concourse import bacc, mybir

    F32 = mybir.dt.float32
    BF16 = mybir.dt.bfloat16
    EXP = mybir.ActivationFunctionType.Exp

    nc = bacc.Bacc("TRN2", target_bir_lowering=False, debug=False)
    xt_d = nc.dram_tensor("xt", [D, S], BF16, kind="ExternalInput")
    wqk_d = nc.dram_tensor("wqk", [D, HPC * DK * 2], BF16,
                           kind="ExternalInput")
    wv_d = nc.dram_tensor("wv", [D, HPC * DV], BF16, kind="ExternalInput")
    out_d = nc.dram_tensor("outT", [HPC * DV, S], F32, kind="ExternalOutput")
    sums_d = nc.dram_tensor("sums", [HPC * NSQ, 512], F32,
                            kind="ExternalOutput")

    with tile.TileContext(nc) as tc, ExitStack() as ctx:
        const_p = ctx.enter_context(tc.tile_pool(name="const", bufs=1))
        xt_p = ctx.enter_context(tc.tile_pool(name="xt", bufs=1))
        w_p = ctx.enter_context(tc.tile_pool(name="w", bufs=1))
        qk_p = ctx.enter_context(tc.tile_pool(name="qk", bufs=1))
        v_p = ctx.enter_context(tc.tile_pool(name="v", bufs=1))
        wt_p = ctx.enter_context(tc.tile_pool(name="wt", bufs=3))
        out_p = ctx.enter_context(tc.tile_pool(name="out", bufs=2))
        import concourse.bass as bass

        psA = ctx.enter_context(
            tc.tile_pool(name="psA", bufs=2, space=bass.MemorySpace.PSUM))
        psB = ctx.enter_context(
            tc.tile_pool(name="psB", bufs=2, space=bass.MemorySpace.PSUM))
        psC = ctx.enter_context(
            tc.tile_pool(name="psC", bufs=2, space=bass.MemorySpace.PSUM))

        ones_f = const_p.tile([P, 1], F32)
        nc.vector.memset(ones_f[:], 1.0)
        ones = const_p.tile([P, 1], BF16)
        nc.vector.tensor_copy(ones[:], ones_f[:])

        xt_sb = xt_p.tile([P, ND * S], BF16)
        wqk_sb = w_p.tile([P, ND * 512], BF16)
        wv_sb = w_p.tile([P, ND * 512], BF16)
        for d in range(ND):
            nc.sync.dma_start(
                xt_sb[:, S * d:S * (d + 1)], xt_d[P * d:P * (d + 1), :])
            nc.sync.dma_start(
                wqk_sb[:, 512 * d:512 * (d + 1)], wqk_d[P * d:P * (d + 1), :])
            nc.sync.dma_start(
                wv_sb[:, 512 * d:512 * (d + 1)], wv_d[P * d:P * (d + 1), :])

        # stage 1a: QT/KT head-pair blocks. blk order: QQ0, KK0, QQ1, KK1.
        # qk_tiles[blk] [128, 2048]: partitions 0-63 = first head of pair,
        # 64-127 = second head.
        qk_tiles = [qk_p.tile([P, S], BF16, name=f"qkt{i}", tag=f"qk{i}")
                    for i in range(4)]
        def emit_qk_blk(blk):
            for sq in range(NSQ):
                ps = psB.tile([P, 512], F32, name="ps1a", tag="b")
                for d in range(ND):
                    nc.tensor.matmul(
                        ps[:],
                        lhsT=wqk_sb[:, 512 * d + P * blk:
                                    512 * d + P * (blk + 1)],
                        rhs=xt_sb[:, S * d + 512 * sq:
                                  S * d + 512 * (sq + 1)],
                        start=(d == 0), stop=(d == ND - 1))
                nc.vector.tensor_copy(
                    qk_tiles[blk][:, 512 * sq:512 * (sq + 1)], ps[:])

        emit_qk_blk(0)
        emit_qk_blk(1)

        # stage 1b: V natural [sk, dv], all 4 heads packed along free dim.
        # vall[:, 512*c + 128*h + v] = V_h[128*c + p, v]
        vall = v_p.tile([P, NSK * 512], BF16)
        for c in range(NSK):
            ps = psC.tile([P, 512], F32, name="ps1b", tag="c")
            for d in range(ND):
                nc.tensor.matmul(
                    ps[:],
                    lhsT=xt_sb[:, S * d + P * c:
                               S * d + P * (c + 1)],
                    rhs=wv_sb[:, 512 * d:512 * (d + 1)],
                    start=(d == 0), stop=(d == ND - 1))
            nc.vector.tensor_copy(vall[:, 512 * c:512 * (c + 1)], ps[:])

        # stage 2: attention per (head-pair, sq-tile), both heads together.
        for pair in range(2):
            if pair == 1:
                emit_qk_blk(2)
                emit_qk_blk(3)
            h0, h1 = 2 * pair, 2 * pair + 1
            qq = qk_tiles[2 * pair]
            kk = qk_tiles[2 * pair + 1]
            for sq in range(NSQ):
                sqs = slice(512 * sq, 512 * (sq + 1))
                av0 = psB.tile([P, 512], F32, name="av0", tag="b")
                av1 = psB.tile([P, 512], F32, name="av1", tag="b")
                ps_sum = psC.tile([P, 512], F32, name="ps_sum", tag="c")

                def emit_av_sums(c, wt):
                    nc.tensor.matmul(
                        av0[:],
                        lhsT=vall[:, 512 * c + P * h0:512 * c + P * (h0 + 1)],
                        rhs=wt[:, 0:512],
                        start=(c == 0), stop=(c == NSK - 1))
                    nc.tensor.matmul(
                        av1[:],
                        lhsT=vall[:, 512 * c + P * h1:512 * c + P * (h1 + 1)],
                        rhs=wt[:, 512:1024],
                        start=(c == 0), stop=(c == NSK - 1))
                    nc.tensor.matmul(
                        ps_sum[0:1, :], lhsT=ones[:], rhs=wt[:, 0:512],
                        start=(c == 0), stop=(c == NSK - 1))
                    nc.tensor.matmul(
                        ps_sum[64:65, :], lhsT=ones[:], rhs=wt[:, 512:1024],
                        start=(c == 0), stop=(c == NSK - 1),
                        tile_position=(0, 64))

                prev = None
                for c in range(NSK):
                    ps_sc = psA.tile([P, 1024], F32, name="ps_sc", tag="a")
                    nc.tensor.matmul(
                        ps_sc[:, 0:512],
                        lhsT=kk[0:DK, P * c:P * (c + 1)],
                        rhs=qq[0:DK, sqs],
                        start=True, stop=True)
                    nc.tensor.matmul(
                        ps_sc[:, 512:1024],
                        lhsT=kk[DK:P, P * c:P * (c + 1)],
                        rhs=qq[DK:P, sqs],
                        start=True, stop=True)
                    wt = wt_p.tile([P, 1024], BF16)
                    nc.scalar.activation(wt[:], ps_sc[:], EXP, scale=0.125)
                    if prev is not None:
                        emit_av_sums(*prev)
                    prev = (c, wt)
                emit_av_sums(*prev)

                sums_sb = out_p.tile([P, 512], F32, name="sums_sb",
                                     tag="s")
                nc.vector.tensor_copy(sums_sb[0:1, :], ps_sum[0:1, :])
                nc.vector.tensor_copy(sums_sb[64:65, :], ps_sum[64:65, :])
                nc.sync.dma_start(sums_d[NSQ * h0 + sq:NSQ * h0 + sq + 1, :],
                                  sums_sb[0:1, :])
                nc.sync.dma_start(sums_d[NSQ * h1 + sq:NSQ * h1 + sq + 1, :],
                                  sums_sb[64:65, :])
                o0 = out_p.tile([P, 512], F32, name="o0", tag="o")
                nc.vector.tensor_copy(o0[:], av0[:])
                nc.sync.dma_start(out_d[P * h0:P * (h0 + 1), sqs], o0[:])
                o1 = out_p.tile([P, 512], F32, name="o1", tag="o")
                nc.vector.tensor_copy(o1[:], av1[:])
                nc.sync.dma_start(out_d[P * h1:P * (h1 + 1), sqs], o1[:])

    nc.compile()
    return nc


def _core_inputs(x, Mq, Mk, Mv, core):
    import ml_dtypes
    bf = ml_dtypes.bfloat16
    b, g = divmod(core, 2)
    hs = [HPC * g + j for j in range(HPC)]
    xt = np.ascontiguousarray(np.asarray(x[b], dtype=np.float32).T).astype(bf)
    cols = []
    for pair in range(2):
        h0, h1 = hs[2 * pair], hs[2 * pair + 1]
        cols += [Mq[:, h0, :], Mq[:, h1, :], Mk[:, h0, :], Mk[:, h1, :]]
    wqk = np.ascontiguousarray(
        np.concatenate(cols, axis=1), dtype=np.float32).astype(bf)
    wv = np.ascontiguousarray(
        np.asarray(Mv[:, hs, :], dtype=np.float32).reshape(
            D, HPC * DV)).astype(bf)
    return {"xt": xt, "wqk": wqk, "wv": wv}


def kernel(x, Mq, Mk, Mv):
    from concourse.bass_utils import run_bass_kernel_spmd

    x = np.asarray(x, dtype=np.float32)
    Mq = np.asarray(Mq, dtype=np.float32)
    Mk = np.asarray(Mk, dtype=np.float32)
    Mv = np.asarray(Mv, dtype=np.float32)

    if "nc" not in _CACHE:
        _CACHE["nc"] = _build_nc()
    nc = _CACHE["nc"]

    in_maps = [_core_inputs(x, Mq, Mk, Mv, c) for c in range(N_CORES)]
    res = run_bass_kernel_spmd(nc, in_maps, list(range(N_CORES))).results

    out = np.empty((B, S, H * DV), dtype=np.float32)
    for core in range(N_CORES):
        b, g = divmod(core, 2)
        outT = res[core]["outT"]                     # [512, 2048] unnormalized
        sums = res[core]["sums"].reshape(HPC, S)     # [4 heads, 2048 sq]
        out[b, :, 512 * g:512 * (g + 1)] = (
            outT.reshape(HPC, DV, S) / sums[:, None, :]).reshape(512, S).T
    return out
